# revision 5
# baseline (speedup 1.0000x reference)
"""Trainium2 Bass kernel for nn_CanadarmJacob (space-arm Jacobian, bm=1 path).

Contract: kernel(**inputs) takes FULL inputs (com_list (512,256,3,7) f32,
link_pose_list (512,256,4,4,9) f32, bm scalar) and returns the FULL output
(512,256,6,7) f32. Internally shards samples across 8 NeuronCores (pure data
parallel), runs a Bass/Tile kernel per core, and gathers.

Math (reformulated from the reference):
  pos   = pose[:3, 3, :7]
  rot   = pose[:3, AXIS[a], a] with AXIS=[2,0,2,2,2,0,2], rot[:,4] *= -1
  del   = com - pos
  jacob = rot x del                          (per-act cross product)
  w_k   = sum_{a>=k} M_a del_a               (suffix mass-weighted cumsum)
  Hphi  = D_suf ⊙ rot + w x jacob            (D_suf = suffix inertia diags)
  S_cc' = sum_a M_a del_c del_c'             (6 unique entries)
  c     = (sum_a M_a com_a)/TM - BASE
  H_s   = TM(c c^T - |c|^2 I) + CD + (Sxx+Syy+Szz) I - S
  jsm   = SM_k jacob_k                       (SM = suffix masses)
  Hth   = Hphi - c x jsm
  A     = -inv(H_s)   (symmetric 3x3, via adjugate and ACT reciprocal)
  bot   = A @ Hth
  top   = -(1/TM) jsm + c x bot
  out   = concat(top, bot) rows
"""
import sys
import functools

if "/opt/trn_rl_repo" not in sys.path:
    sys.path.insert(0, "/opt/trn_rl_repo")

import numpy as np

# ---------------------------------------------------------------- constants
N_CORES = 8
P = 128          # SBUF partitions
J = 128          # samples per partition per core
S_CORE = P * J   # 16384 samples per core
N_ACT = 7

MASS = np.array([105.98, 105.98, 314.98, 279.2, 105.98, 105.98, 243.66], np.float64)
TM = float(MASS.sum() + 100000.0 + 243.66)
DIAGS = np.array([[12.19, 12.19, 3.061], [12.19, 12.19, 3.061], [15.41, 2094.71, 2103.19],
                  [9.522, 1966.28, 1966.28], [8.305, 3.061, 8.0386], [12.13, 12.13, 3.061],
                  [9.336, 44.41, 44.41]], np.float64)
D_SUF = np.cumsum(DIAGS[::-1], axis=0)[::-1]          # (7,3) suffix inertia diag
SM = np.cumsum(MASS[::-1])[::-1]                      # (7,) suffix mass
CD = DIAGS.sum(axis=0)                                # (3,)
_TF0 = np.array([[1, 0, 0, 0], [0, -1, 0, 0], [0, 0, 1.3, 6], [0, 0, 0, 1]], np.float64)
_COM0 = np.array([[1, 0, 0, 0], [0, 1, 0, 0], [0, 0, 1, 0.5], [0, 0, 0, 1]], np.float64)
BASE = (_TF0 @ _COM0)[:3, 3] * 243.66 / (100000.0 + 243.66)   # [0, 0, ~0.0162]

# consts row layout (replicated to all 128 partitions host-side)
#   [0:7] M, [7:14] SM, [14:35] D (c-major: D[c][a]), [35:38] CD
CONSTS = np.concatenate([MASS, SM, D_SUF.T.reshape(-1), CD]).astype(np.float32)
NCONST = CONSTS.shape[0]

# smalls tile row indices (each row is (128, J) f32)
SS_R, CSQ_R = 0, 1
CC = 2            # rows 2..4 diag (xx,yy,zz), 5..7 off (xy,xz,yz)
HS = 8            # rows 8..13: [xx,yy,zz,xy,xz,yz]
ADJ = 14          # rows 14..19: [a11,a22,a33,a12,a13,a23]
M1_R, M2_R = 20, 22
T0_R, T1_R, T2_R = 24, 25, 26
DET_R, RDN_R = 27, 28
NSMALL = 29


def _emit(nc, tc, ctx, dram):
    import concourse.bass as bass
    from concourse import mybir

    f32 = mybir.dt.float32
    OP = mybir.AluOpType
    AX = mybir.AxisListType
    V = nc.vector
    G = nc.gpsimd

    pool = ctx.enter_context(tc.tile_pool(name="main", bufs=1))

    consts = pool.tile([P, NCONST], f32)
    pose = pool.tile([P, J, 108], f32, tag="pose")
    com = pool.tile([P, J, 21], f32, tag="com")
    rot = pool.tile([P, 3, J, N_ACT], f32)
    delb = pool.tile([P, 3, J, N_ACT], f32, tag="dj")
    mw = pool.tile([P, 3, J, N_ACT], f32)     # mdel, suffix-summed in place -> w
    jac = pool.tile([P, 3, J, N_ACT], f32)
    hphi = pool.tile([P, 3, J, N_ACT], f32)   # Hphi -> Htheta in place
    prod = pool.tile([P, 9, J, N_ACT], f32)   # 6 S-products + 3 mcom; later scratch
    cv = pool.tile([P, 3, J], f32)
    smalls = pool.tile([P, NSMALL, J], f32)
    outb = pool.tile([P, J, 42], f32, tag="pose")   # reuses pose slot
    red = pool.tile([P, 9, J], f32, tag="com")      # reuses com slot

    # input DMAs (HWDGE)
    nc.sync.dma_start(out=consts[:], in_=dram["consts"][:])
    nc.sync.dma_start(out=com[:], in_=dram["com"][:])
    nc.sync.dma_start(out=pose[:], in_=dram["pose"][:, :, 0:108])

    # handy views
    comR = com[:].rearrange("p j (c a) -> p c j a", c=3)          # (P,3,J,7)
    poseR = pose[:].rearrange("p j (r k) -> p r j k", r=3)        # (P,3,J,36)
    posV = poseR[:, :, :, 27:34]                                  # (P,3,J,7)

    def bc(ap, shape):
        return ap.broadcast_to(shape)

    Mb = bc(consts[:, 0:7].unsqueeze(1).unsqueeze(2), (P, 3, J, N_ACT))
    SMb = bc(consts[:, 7:14].unsqueeze(1).unsqueeze(2), (P, 3, J, N_ACT))
    Db = bc(consts[:, 14:35].rearrange("p (c a) -> p c a", c=3).unsqueeze(2),
            (P, 3, J, N_ACT))
    CDb = bc(consts[:, 35:38].unsqueeze(2), (P, 3, J))

    # rot gather from pose (GPSIMD, off the DVE critical path)
    G.tensor_copy(out=rot[:], in_=poseR[:, :, :, 18:25])          # axis col 2
    G.tensor_copy(out=rot[:, :, :, 1:6:4], in_=poseR[:, :, :, 1:6:4])  # acts 1,5 from col 0
    G.tensor_scalar_mul(rot[:, :, :, 4], rot[:, :, :, 4], -1.0)   # sign flip act 4

    # mcom products -> prod rows 6..8 ; del ; mdel ; S-products
    V.tensor_tensor(out=prod[:, 6:9], in0=Mb, in1=comR, op=OP.mult)
    V.tensor_tensor(out=delb[:], in0=comR, in1=posV, op=OP.subtract)
    V.tensor_tensor(out=mw[:], in0=Mb, in1=delb[:], op=OP.mult)
    for k, (i, j) in enumerate([(0, 0), (1, 1), (2, 2), (0, 1), (0, 2), (1, 2)]):
        V.tensor_tensor(out=prod[:, k], in0=mw[:, i], in1=delb[:, j], op=OP.mult)

    # one big reduction over acts: (P,9,J,7) -> (P,9,J)
    V.tensor_reduce(out=red[:], in_=prod[:], axis=AX.X, op=OP.add)

    # jacob = rot x del  (prod rows now scratch)
    tu = prod[:, 0:3]
    tv = prod[:, 3:6]
    for cx in range(3):
        y, z = (cx + 1) % 3, (cx + 2) % 3
        V.tensor_tensor(out=tu[:, cx], in0=rot[:, y], in1=delb[:, z], op=OP.mult)
        V.tensor_tensor(out=tv[:, cx], in0=rot[:, z], in1=delb[:, y], op=OP.mult)
        V.tensor_tensor(out=jac[:, cx], in0=tu[:, cx], in1=tv[:, cx], op=OP.subtract)

    # suffix cumsum over acts in place: mw becomes w
    for k in range(5, -1, -1):
        V.tensor_tensor(out=mw[:, :, :, k], in0=mw[:, :, :, k], in1=mw[:, :, :, k + 1],
                        op=OP.add)

    # jsm = SM * jacob (reuses delb slot via tag)
    jsm = delb  # overwritten after last delb read (jacob products)
    V.tensor_tensor(out=jsm[:], in0=SMb, in1=jac[:], op=OP.mult)

    # Hphi = D*rot + w x jacob
    for cx in range(3):
        y, z = (cx + 1) % 3, (cx + 2) % 3
        V.tensor_tensor(out=tu[:, cx], in0=mw[:, y], in1=jac[:, z], op=OP.mult)
        V.tensor_tensor(out=tv[:, cx], in0=mw[:, z], in1=jac[:, y], op=OP.mult)
        V.tensor_tensor(out=hphi[:, cx], in0=tu[:, cx], in1=tv[:, cx], op=OP.subtract)
    V.tensor_tensor(out=tu[:], in0=rot[:], in1=Db, op=OP.mult)
    V.tensor_tensor(out=hphi[:], in0=hphi[:], in1=tu[:], op=OP.add)

    # c = scom/TM - BASE   (BASE is [0,0,bz])
    V.tensor_scalar(out=cv[:, 0:2], in0=red[:, 6:8], scalar1=1.0 / TM, scalar2=None,
                    op0=OP.mult)
    V.tensor_scalar(out=cv[:, 2], in0=red[:, 8], scalar1=1.0 / TM,
                    scalar2=float(BASE[2]), op0=OP.mult, op1=OP.subtract)

    # cc products and |c|^2, SS
    V.tensor_tensor(out=smalls[:, CC:CC + 3], in0=cv[:], in1=cv[:], op=OP.mult)
    for k, (i, j) in enumerate([(0, 1), (0, 2), (1, 2)]):
        V.tensor_tensor(out=smalls[:, CC + 3 + k], in0=cv[:, i], in1=cv[:, j],
                        op=OP.mult)
    V.tensor_reduce(out=smalls[:, SS_R], in_=red[:, 0:3].transpose([0, 2, 1]),
                    axis=AX.X, op=OP.add)
    V.tensor_reduce(out=smalls[:, CSQ_R], in_=smalls[:, CC:CC + 3].transpose([0, 2, 1]),
                    axis=AX.X, op=OP.add)

    csq_b = bc(smalls[:, CSQ_R].unsqueeze(1), (P, 3, J))
    ss_b = bc(smalls[:, SS_R].unsqueeze(1), (P, 3, J))

    # H_s diag rows HS..HS+2 ; off rows HS+3..HS+5
    a1 = smalls[:, M1_R:M1_R + 2]  # scratch pair rows (reused a lot below)
    V.tensor_tensor(out=smalls[:, T0_R:T0_R + 3], in0=smalls[:, CC:CC + 3], in1=csq_b,
                    op=OP.subtract)
    V.tensor_tensor(out=smalls[:, HS:HS + 3], in0=ss_b, in1=red[:, 0:3], op=OP.subtract)
    nc.vector.scalar_tensor_tensor(out=smalls[:, HS:HS + 3], in0=smalls[:, T0_R:T0_R + 3],
                                   scalar=TM, in1=smalls[:, HS:HS + 3],
                                   op0=OP.mult, op1=OP.add)
    V.tensor_tensor(out=smalls[:, HS:HS + 3], in0=smalls[:, HS:HS + 3], in1=CDb,
                    op=OP.add)
    nc.vector.scalar_tensor_tensor(out=smalls[:, HS + 3:HS + 6],
                                   in0=smalls[:, CC + 3:CC + 6], scalar=TM,
                                   in1=red[:, 3:6], op0=OP.mult, op1=OP.subtract)

    # adjugate (batched pairs via reversed/broadcast row views)
    h = lambda i: smalls[:, HS + i]
    hpair = lambda a, b: smalls[:, HS + a: (HS + b - 1 if b < a else HS + b + 1): (1 if b > a else -1)]
    b2 = lambda ap: bc(ap.unsqueeze(1), (P, 2, J))
    # a11 = h1 h2 - h5^2 ; a22 = h0 h2 - h4^2
    V.tensor_tensor(out=smalls[:, M1_R:M1_R + 2], in0=hpair(1, 0), in1=b2(h(2)), op=OP.mult)
    V.tensor_tensor(out=smalls[:, M2_R:M2_R + 2], in0=hpair(5, 4), in1=hpair(5, 4), op=OP.mult)
    V.tensor_tensor(out=smalls[:, ADJ:ADJ + 2], in0=smalls[:, M1_R:M1_R + 2],
                    in1=smalls[:, M2_R:M2_R + 2], op=OP.subtract)
    # a33 = h0 h1 - h3^2
    V.tensor_tensor(out=smalls[:, T0_R], in0=h(0), in1=h(1), op=OP.mult)
    V.tensor_tensor(out=smalls[:, T1_R], in0=h(3), in1=h(3), op=OP.mult)
    V.tensor_tensor(out=smalls[:, ADJ + 2], in0=smalls[:, T0_R], in1=smalls[:, T1_R],
                    op=OP.subtract)
    # a12 = h4 h5 - h3 h2 ; a13 = h3 h5 - h4 h1
    V.tensor_tensor(out=smalls[:, M1_R:M1_R + 2], in0=hpair(4, 3), in1=b2(h(5)), op=OP.mult)
    V.tensor_tensor(out=smalls[:, M2_R:M2_R + 2], in0=hpair(3, 4), in1=hpair(2, 1), op=OP.mult)
    V.tensor_tensor(out=smalls[:, ADJ + 3:ADJ + 5], in0=smalls[:, M1_R:M1_R + 2],
                    in1=smalls[:, M2_R:M2_R + 2], op=OP.subtract)
    # a23 = h3 h4 - h0 h5
    V.tensor_tensor(out=smalls[:, T0_R], in0=h(3), in1=h(4), op=OP.mult)
    V.tensor_tensor(out=smalls[:, T1_R], in0=h(0), in1=h(5), op=OP.mult)
    V.tensor_tensor(out=smalls[:, ADJ + 5], in0=smalls[:, T0_R], in1=smalls[:, T1_R],
                    op=OP.subtract)

    # det = h0 a11 + h3 a12 + h4 a13 ; A = adj * (-1/det)
    V.tensor_tensor(out=smalls[:, T0_R], in0=h(0), in1=smalls[:, ADJ], op=OP.mult)
    V.tensor_tensor(out=smalls[:, T1_R], in0=h(3), in1=smalls[:, ADJ + 3], op=OP.mult)
    V.tensor_tensor(out=smalls[:, T2_R], in0=h(4), in1=smalls[:, ADJ + 4], op=OP.mult)
    V.tensor_tensor(out=smalls[:, DET_R], in0=smalls[:, T0_R], in1=smalls[:, T1_R],
                    op=OP.add)
    V.tensor_tensor(out=smalls[:, DET_R], in0=smalls[:, DET_R], in1=smalls[:, T2_R],
                    op=OP.add)
    V.reciprocal(out=smalls[:, RDN_R], in_=smalls[:, DET_R])
    rdn_b = bc(smalls[:, RDN_R].unsqueeze(1), (P, 6, J))
    nc.vector.scalar_tensor_tensor(out=smalls[:, ADJ:ADJ + 6],
                                   in0=smalls[:, ADJ:ADJ + 6], scalar=-1.0,
                                   in1=rdn_b, op0=OP.mult, op1=OP.mult)

    cvb = lambda i: bc(cv[:, i].unsqueeze(2), (P, J, N_ACT))

    # Htheta = Hphi - c x jsm  (in place on hphi)
    for cx in range(3):
        y, z = (cx + 1) % 3, (cx + 2) % 3
        V.tensor_tensor(out=tu[:, cx], in0=cvb(y), in1=jsm[:, z], op=OP.mult)
        V.tensor_tensor(out=hphi[:, cx], in0=hphi[:, cx], in1=tu[:, cx], op=OP.subtract)
        V.tensor_tensor(out=tv[:, cx], in0=cvb(z), in1=jsm[:, y], op=OP.mult)
        V.tensor_tensor(out=hphi[:, cx], in0=hphi[:, cx], in1=tv[:, cx], op=OP.add)

    # bot = A @ Htheta  -> outb cols 21..41
    Arows = [[0, 3, 4], [3, 1, 5], [4, 5, 2]]
    Ab = lambda r: bc(smalls[:, ADJ + r].unsqueeze(2), (P, J, N_ACT))
    bot = lambda c: outb[:, :, 21 + 7 * c: 28 + 7 * c]
    for oc in range(3):
        r0, r1, r2 = Arows[oc]
        V.tensor_tensor(out=tu[:, 0], in0=Ab(r0), in1=hphi[:, 0], op=OP.mult)
        V.tensor_tensor(out=tu[:, 1], in0=Ab(r1), in1=hphi[:, 1], op=OP.mult)
        V.tensor_tensor(out=tu[:, 2], in0=tu[:, 0], in1=tu[:, 1], op=OP.add)
        V.tensor_tensor(out=tu[:, 0], in0=Ab(r2), in1=hphi[:, 2], op=OP.mult)
        V.tensor_tensor(out=bot(oc), in0=tu[:, 2], in1=tu[:, 0], op=OP.add)

    # top = -(1/TM) jsm + c x bot -> outb cols 0..20
    for cx in range(3):
        y, z = (cx + 1) % 3, (cx + 2) % 3
        V.tensor_tensor(out=tu[:, cx], in0=cvb(y), in1=bot(z), op=OP.mult)
        nc.vector.scalar_tensor_tensor(out=tv[:, cx], in0=jsm[:, cx],
                                       scalar=-1.0 / TM, in1=tu[:, cx],
                                       op0=OP.mult, op1=OP.add)
        V.tensor_tensor(out=tu[:, cx], in0=cvb(z), in1=bot(y), op=OP.mult)
        V.tensor_tensor(out=outb[:, :, 7 * cx: 7 * cx + 7], in0=tv[:, cx],
                        in1=tu[:, cx], op=OP.subtract)

    nc.sync.dma_start(out=dram["out"][:], in_=outb[:])


@functools.lru_cache(maxsize=1)
def _program():
    from contextlib import ExitStack
    import concourse.bacc as bacc
    import concourse.tile as tile
    from concourse import mybir

    f32 = mybir.dt.float32
    nc = bacc.Bacc("TRN2", target_bir_lowering=False, debug=False)
    dram = {
        "com": nc.dram_tensor("com", [P, J, 21], f32, kind="ExternalInput"),
        "pose": nc.dram_tensor("pose", [P, J, 144], f32, kind="ExternalInput"),
        "consts": nc.dram_tensor("consts", [P, NCONST], f32, kind="ExternalInput"),
        "out": nc.dram_tensor("out", [P, J, 42], f32, kind="ExternalOutput"),
    }
    with tile.TileContext(nc) as tc:
        with ExitStack() as ctx:
            _emit(nc, tc, ctx, dram)
    nc.compile()
    return nc


def _kernel_bm0(com, pose):
    # bm=0 path (not exercised by the shipped setup_inputs; numpy fallback)
    rot = pose[:, :, :3, 2, :N_ACT].copy()
    rot[..., 1] = pose[:, :, :3, 0, 1]
    rot[..., 5] = pose[:, :, :3, 0, 5]
    rot[..., 4] *= -1.0
    delp = pose[:, :, :3, 3, -2][..., None] - pose[:, :, :3, 3, :N_ACT]
    jt = np.cross(rot, delp, axis=2)
    return np.concatenate([jt, rot], axis=2).astype(np.float32)


def kernel(com_list, link_pose_list, bm):
    com_list = np.ascontiguousarray(com_list, dtype=np.float32)
    link_pose_list = np.ascontiguousarray(link_pose_list, dtype=np.float32)
    if not int(bm):
        return _kernel_bm0(com_list, link_pose_list)

    from concourse.bass_utils import run_bass_kernel_spmd

    nc = _program()
    com_flat = com_list.reshape(N_CORES, P, J, 21)
    pose_flat = link_pose_list.reshape(N_CORES, P, J, 144)
    consts = np.broadcast_to(CONSTS, (P, NCONST)).copy()
    in_maps = [
        {"com": com_flat[k], "pose": pose_flat[k], "consts": consts}
        for k in range(N_CORES)
    ]
    res = run_bass_kernel_spmd(nc, in_maps, core_ids=list(range(N_CORES)))
    out = np.stack([res.results[k]["out"] for k in range(N_CORES)])
    return out.reshape(512, 256, 6, 7)
